# revision 3
# baseline (speedup 1.0000x reference)
"""Trainium2 Bass kernel: GNN conv block (nn_Conv_block_49331994362308).

Computes, for N=100000 nodes with K=16 neighbors each:
    nh  = ij[:, :, 0]                      # [N, K] neighbor ids
    xnj = mean(x[nh], axis=1)              # neighbor-feature mean  [N, 128]
    xej = mean(e, axis=1)                  # edge-feature mean      [N, 64]
    out = relu(x @ Wc.T + xnj @ Wn.T + xej @ We.T)

Distribution: data-parallel over nodes across 8 NeuronCores (12500 nodes
per core, padded to 12544 = 98*128). x is replicated to every core so the
random neighbor gather x[nh] is a core-local indirect DMA from HBM.

The kernel is DMA-bytes-bound (~435 GB/s per-core ceiling), so all bulk
streams are 16-bit or built on-device:
  - Neighbor rows are gathered from a bf16 copy of x (256B per row — the
    dma_gather minimum element size) via InstDMAGatherAnt, one instruction
    per mod-4 row class per 7-tile group (~4480 rows each; the SWDGE path
    costs ~1us fixed per instruction). Indices are int16, so x is viewed
    as [N/4, 4, 128] super-rows; the host buckets each tile's 2048 edges
    by nh%4, pads each bucket to 640 slots (actual max 591 for this
    dataset), and emits nh//4 as the index stream. Gathered bf16 rows
    feed PE pool matmuls directly — no cast pass.
  - The one-hot pooling matrices P[slot, node] are built ON DEVICE (fp8)
    by one broadcast DVE is_equal per tile against a host-sent owner
    vector [128 slot-partitions, 20 chunks] — replacing a 32MB/core HBM
    stream with a ~0.5MB one.
  - e is sent bf16 and feature-major [64, nodes*K] so the DVE mean-reduce
    is contiguous over K and lands pre-transposed for the final matmul.
  - x_self is sent pre-transposed bf16 ([128 feat, nodes]) so the self
    term needs no PE transpose and no PSUM round-trip.
  - Final 3 accumulating matmuls are bf16 (stationaries xT / xnjT / xejT,
    movers the bf16 weights with 1/K pre-folded); ACT copies xnjT
    PSUM->SBUF (with bf16 cast), DVE applies ReLU into a bf16 staging
    buffer flushed to DRAM once per 14-tile chunk; the host upcasts.

Walrus's TRN2 queue-DMA codegen only supports ONE sync-wait command per
DMA (and one per PE LDWEIGHTS), so the structure keeps every DMA at a
single dependency front: indices are preloaded once into SBUF (gathers
then wait only on the PE pool-slot release), the 8 SWDGE bookkeeping
lanes are warmed with dummy transfers that absorb the preload front, and
outputs go to once-written per-chunk DRAM tensors (no WAW chains).
"""

from contextlib import ExitStack

import numpy as np

import concourse.bass as bass
import concourse.mybir as mybir
import concourse.tile as tile
from concourse.bass_utils import run_bass_kernel_spmd
from concourse import library_config

P = 128
K = 16
XN_IN = 128
XE_IN = 64
XN_OUT = 128
N_CORES = 8
N_FULL = 100000
N_LOC = N_FULL // N_CORES          # 12500
N_LOC_PAD = ((N_LOC + P - 1) // P) * P  # 12544
CHUNK = 14                          # tiles per output chunk (98 = 7*14)

F32 = mybir.dt.float32
BF16 = mybir.dt.bfloat16
F8 = mybir.dt.float8e4   # pooling matrices hold only 0/1 — exact in fp8
I16 = mybir.dt.int16

GRP = 7            # tiles per gather group
NCLS = 4           # x rows per int16 "super-row" (mod classes)
SEG = 640          # padded gather slots per (tile, class); 5 chunks of 128
NCH = SEG // P     # pool chunks per (tile, class) = 5
CH_T = NCH * NCLS  # pool chunks per tile = 20


def _chunks(n_tiles: int) -> list[int]:
    out = []
    t = 0
    while t < n_tiles:
        out.append(min(CHUNK, n_tiles - t))
        t += CHUNK
    return out


def build_program(n_loc_pad: int, n_src: int) -> bass.Bass:
    """Build the SPMD per-core Bass program (same program on every core)."""
    assert n_loc_pad % P == 0
    n_tiles = n_loc_pad // P
    chunks = _chunks(n_tiles)

    # detect_race_conditions=False: the post-schedule wait-legalizer's nop
    # carriers share scratch tiles and trip the sim race detector's
    # bookkeeping (same-engine program order makes them safe).
    nc = bass.Bass("TRN2", debug=False, detect_race_conditions=False)

    assert n_tiles % GRP == 0
    n_groups = n_tiles // GRP
    seg_i16 = GRP * SEG // 16  # idx16 columns per (group, class)

    x_bf = nc.dram_tensor("x_bf", [n_src, XN_IN], BF16, kind="ExternalInput").ap()
    # x_self pre-transposed: [128 feat, nodes] bf16
    x_selfT = nc.dram_tensor("x_selfT", [XN_IN, n_loc_pad], BF16, kind="ExternalInput").ap()
    # e feature-major: [64 feat, nodes*K] bf16, k contiguous per node
    e_locT = nc.dram_tensor("e_locT", [XE_IN, n_loc_pad * K], BF16, kind="ExternalInput").ap()
    # int16 super-row ids (nh//4), wrapped [16, L/16] + replicated to 128
    # partitions, concatenated over (group, class)
    idx_loc = nc.dram_tensor(
        "idx_loc", [P, n_groups * NCLS * seg_i16], I16, kind="ExternalInput"
    ).ap()
    # slot owners per tile: [128 slot-partitions, n_tiles*CH_T] bf16
    # (node id 0..127 within tile, or -1 for padding slots)
    owner_loc = nc.dram_tensor(
        "owner_loc", [P, n_tiles * CH_T], BF16, kind="ExternalInput"
    ).ap()
    iota_in = nc.dram_tensor("iota_in", [P, P], BF16, kind="ExternalInput").ap()
    wcT = nc.dram_tensor("wcT", [XN_IN, XN_OUT], BF16, kind="ExternalInput").ap()
    wnT = nc.dram_tensor("wnT", [XN_IN, XN_OUT], BF16, kind="ExternalInput").ap()
    weT = nc.dram_tensor("weT", [XE_IN, XN_OUT], BF16, kind="ExternalInput").ap()
    # per-chunk outputs, partition-major: out_c[p, i*128+f] = out[(t0+i)*128+p, f]
    outs = [
        nc.dram_tensor(f"out{c}", [P, ct * XN_OUT], BF16, kind="ExternalOutput").ap()
        for c, ct in enumerate(chunks)
    ]

    nop_sem = nc.alloc_semaphore("waitnop")

    with tile.TileContext(nc) as tc, ExitStack() as ctx:
        nc.gpsimd.sem_clear(range(nop_sem.num, nop_sem.num + 1))
        nc.gpsimd.load_library(library_config.mlp)
        consts = ctx.enter_context(tc.tile_pool(name="consts", bufs=1))
        wcT_sb = consts.tile([XN_IN, XN_OUT], BF16, tag="wc")
        wnT_sb = consts.tile([XN_IN, XN_OUT], BF16, tag="wn")
        weT_sb = consts.tile([XE_IN, XN_OUT], BF16, tag="we")
        iota_sb = consts.tile([P, P], BF16, tag="iota")
        nc.sync.dma_start(wcT_sb[:], wcT[:, :])
        nc.sync.dma_start(wnT_sb[:], wnT[:, :])
        nc.sync.dma_start(weT_sb[:], weT[:, :])
        nc.sync.dma_start(iota_sb[:], iota_in[:, :])
        idx_all = consts.tile([P, n_groups * NCLS * seg_i16], I16, tag="idx_all")
        nc.sync.dma_start(idx_all[:], idx_loc[:, :])
        # x viewed as [n_src/4, 4, 128]: class j gathers row 4*i16+j via
        # elem_step=512 elements (1024B stride) and a j*128-element offset
        x4 = x_bf.rearrange("(r c) f -> r c f", c=NCLS)

        # Warm the 8 SWDGE bookkeeping lanes: each dummy absorbs the
        # idx-preload front so later gathers carry only their PE front.
        scratch = ctx.enter_context(tc.tile_pool(name="scratch", bufs=1))
        for q in range(8):
            sc = scratch.tile([1, K], I16, tag=f"sc{q}")
            nc.gpsimd.dma_start(sc[:], idx_all[:1, :K])
        # Tiny template instructions for _legalize_waits nop carriers
        # (one per DMA queue and per compute engine).
        nop_hw = scratch.tile([1, K], I16, tag="noptpl_hw")
        nc.sync.dma_start(nop_hw[:], idx_loc[:1, :K])
        nop_sw = scratch.tile([1, K], I16, tag="noptpl_sw")
        nc.gpsimd.dma_start(nop_sw[:], idx_loc[:1, :K])
        nop_dve = scratch.tile([P, K], BF16, tag="noptpl_dve")
        nc.vector.tensor_copy(nop_dve[:], iota_sb[:, :K])
        nop_act = scratch.tile([P, K], BF16, tag="noptpl_act")
        nc.scalar.copy(nop_act[:], iota_sb[:, :K])
        nop_pool = scratch.tile([P, K], F32, tag="noptpl_pool")
        nc.gpsimd.memset(nop_pool[:], 0.0)

        g_pool = ctx.enter_context(tc.tile_pool(name="gatherp", bufs=2))
        pm_pool = ctx.enter_context(tc.tile_pool(name="pmatp", bufs=3))
        ow_pool = ctx.enter_context(tc.tile_pool(name="ownp", bufs=3))
        e_pool = ctx.enter_context(tc.tile_pool(name="edgep", bufs=4))
        xs_pool = ctx.enter_context(tc.tile_pool(name="xselfp", bufs=4))
        st_pool = ctx.enter_context(tc.tile_pool(name="stagep", bufs=3))
        out_pool = ctx.enter_context(tc.tile_pool(name="outp", bufs=2))
        psum_pool = ctx.enter_context(tc.tile_pool(name="psump", bufs=2, space="PSUM"))
        psum1_pool = ctx.enter_context(tc.tile_pool(name="psum1p", bufs=1, space="PSUM"))

        # Warm up PE's view of the constants so steady-state matmuls carry at
        # most one sync wait (PE LDWEIGHTS supports a single wait command).
        ps_warm = psum1_pool.tile([P, P], F32, tag="warm")
        nc.tensor.matmul(ps_warm[:], iota_sb[:], iota_sb[:], start=True, stop=False)
        nc.tensor.matmul(ps_warm[:], wcT_sb[:], wcT_sb[:], start=False, stop=False)
        nc.tensor.matmul(ps_warm[:], wnT_sb[:], wnT_sb[:], start=False, stop=False)
        nc.tensor.matmul(
            ps_warm[:], weT_sb[:], iota_sb[:XE_IN, :], start=False, stop=True
        )

        t = 0
        gbf = [None] * NCLS
        nidx_reg = nc.gpsimd.to_reg(GRP * SEG)  # shared across all gathers
        for c, ct in enumerate(chunks):
            o_stage = out_pool.tile([P, ct * XN_OUT], BF16, tag="ostage")
            for i in range(ct):
                g, ti = divmod(t, GRP)

                if ti == 0:
                    # per-group gathers: one dma_gather per mod-4 class of
                    # GRP*SEG slots; slot i lands at partition i%128, free
                    # block i//128, so 128-slot chunks stay within one tile.
                    for j in range(NCLS):
                        off = (g * NCLS + j) * seg_i16
                        gb = g_pool.tile(
                            [P, GRP * SEG // P, XN_IN], BF16, tag=f"go{j}"
                        )
                        nc.gpsimd.dma_gather(
                            out_ap=gb[:],
                            in_ap=x4[:, j, :],
                            idxs_ap=idx_all[:, off:off + seg_i16],
                            num_idxs=GRP * SEG,
                            num_idxs_reg=nidx_reg,
                            elem_size=XN_IN,
                            elem_step=NCLS * XN_IN,
                            single_packet=False,
                        )
                        gbf[j] = gb

                x_sb = xs_pool.tile([XN_IN, P], BF16, tag="xs")
                nc.sync.dma_start(x_sb[:], x_selfT[:, t * P:(t + 1) * P])
                e_sb = e_pool.tile([XE_IN, P * K], BF16, tag="e")
                nc.sync.dma_start(e_sb[:], e_locT[:, t * P * K:(t + 1) * P * K])
                own_sb = ow_pool.tile([P, CH_T], BF16, tag="own")
                nc.sync.dma_start(
                    own_sb[:], owner_loc[:, t * CH_T:(t + 1) * CH_T]
                )

                # P[slot, b*128+n] = (owner[slot, b] == n), one broadcast
                # is_equal on DVE per tile (fp8 out; 0/1 exact)
                pmat = pm_pool.tile([P, CH_T, P], F8, tag="pmat")
                own_bc = own_sb[:].rearrange("p (c o) -> p c o", o=1).broadcast_to(
                    [P, CH_T, P]
                )
                iota_bc = iota_sb[:].rearrange("p (o n) -> p o n", o=1).broadcast_to(
                    [P, CH_T, P]
                )
                nc.vector.tensor_tensor(
                    pmat[:], own_bc, iota_bc, mybir.AluOpType.is_equal
                )

                # xnjT[f, n] = sum_slot g[slot, f] * P[slot, n]
                xnjT_ps = psum_pool.tile([P, P], F32, tag="ps_xnj")
                for b in range(CH_T):
                    j, bl = divmod(b, NCH)
                    blk = ti * NCH + bl
                    nc.tensor.matmul(
                        xnjT_ps[:],
                        gbf[j][:, blk, :],
                        pmat[:, b, :],
                        start=(b == 0),
                        stop=(b == CH_T - 1),
                    )

                # xejT[f, n] = sum_k e[f, n, k] on DVE (contiguous k)
                xejT_sb = st_pool.tile([XE_IN, P], BF16, tag="xej")
                e_view = e_sb[:].rearrange("f (n k) -> f n k", k=K)
                with nc.allow_low_precision(
                    reason="sum of 16 bf16 edge features; 2e-2 gate"
                ):
                    nc.vector.tensor_reduce(
                        xejT_sb[:], e_view, axis=mybir.AxisListType.X,
                        op=mybir.AluOpType.add,
                    )

                # xnjT PSUM -> SBUF (+ bf16 cast) on ACT
                xnjT_sb = st_pool.tile([P, P], BF16, tag="sb_xnj")
                nc.scalar.copy(xnjT_sb[:], xnjT_ps[:])

                out_ps = psum_pool.tile([P, XN_OUT], F32, tag="ps_out")
                nc.tensor.matmul(out_ps[:], x_sb[:], wcT_sb[:], start=True, stop=False)
                nc.tensor.matmul(out_ps[:], xnjT_sb[:], wnT_sb[:], start=False, stop=False)
                nc.tensor.matmul(out_ps[:], xejT_sb[:], weT_sb[:], start=False, stop=True)

                # ReLU on DVE into the chunk staging buffer (PSUM releases all
                # flow through the one DVE semaphore PE already waits on).
                nc.vector.tensor_scalar_max(
                    o_stage[:, i * XN_OUT:(i + 1) * XN_OUT], out_ps[:], 0.0
                )
                t += 1

            nc.sync.dma_start(outs[c][:, :], o_stage[:])

    from concourse.library_overlay import lower_extended_insts

    lower_extended_insts(nc)
    _legalize_waits(nc, nop_sem)
    return nc


def _legalize_waits(nc: bass.Bass, nop_sem) -> None:
    """Split multi-wait queue-DMAs / matmuls for walrus's 1-wait codegen limit.

    The TRN2 walrus codegen allows a single sync-wait command per queue-DMA
    entry and per PE matmul (S3_LW struct). Tile emits minimal waits but can
    still produce 2+ (e.g. a slot's previous-writer DMA completion plus its
    last-reader engine release — Tile's clocks are not transitive). Queue
    entries execute in FIFO order, so extra waits are moved onto tiny no-op
    carrier DMAs inserted immediately before the offender on the same queue.
    For matmuls the carrier is a 1-column bf16 LDWEIGHTS (any clobbered
    weights are reloaded by each matmul's own weight load; insertion happens
    before a directly-preceding LDWEIGHTS so split LDW+MM pairs stay intact).
    """
    import copy

    dma_tpl: dict = {}
    eng_tpl: dict = {}
    evsem_tpl: dict = {}
    ldw_tpl = None
    for f in nc.m.functions:
        for blk in f.blocks:
            for inst in blk.instructions:
                tn = type(inst).__name__
                dst = (
                    str(getattr(inst.outs[0], "memref", "")) if inst.outs else ""
                )
                if tn == "InstDMACopy":
                    if dst.startswith("nop_hw"):
                        dma_tpl["qSPDynamicHW"] = inst
                    elif dst.startswith("nop_sw"):
                        dma_tpl[inst.queue] = inst
                elif tn == "InstLdweights" and ldw_tpl is None:
                    ldw_tpl = inst
                elif tn == "InstEventSemaphore":
                    evsem_tpl[inst.engine] = inst
                elif dst.startswith("nop_dve") or dst.startswith("nop_act") or dst.startswith("nop_pool"):
                    eng_tpl[inst.engine] = inst

    counter = [0]

    def make_nop(tpl, wait):
        counter[0] += 1
        nop = copy.deepcopy(tpl)
        nop.name = f"I-{nc.next_id()}"
        # DMA carriers must update a semaphore (BIR invariant); use a
        # dedicated one nobody waits on. Other engines' carriers stay
        # update-free (walrus rejects a waitnop update on e.g. TensorCopy
        # with a no_semaphore_value_conflict ISA check).
        upd = []
        if type(tpl).__name__ == "InstDMACopy":
            upd = [
                mybir.SyncUpdate(
                    sync_type="semaphore",
                    id=nop_sem.num,
                    ant_name=nop_sem.name,
                    update_mode="sem-add-imm",
                    update_value=16,
                )
            ]
        nop.sync_info = mybir.SyncInfo(on_wait=[wait], on_update=upd)
        nc.inst_map[nop.name] = nop
        return nop

    for f in nc.m.functions:
        for blk in f.blocks:
            out: list = []
            changed = False
            insts = list(blk.instructions)
            for pos, inst in enumerate(insts):
                tn = type(inst).__name__
                si = inst.sync_info
                waits = list(si.on_wait) if si else []
                nops = None
                if len(waits) > 1:
                    if tn == "InstDMACopy":
                        tpl = dma_tpl.get(inst.queue)
                        assert tpl is not None, f"no nop template for {inst.queue}"
                        nops = [make_nop(tpl, w) for w in waits[:-1]]
                    elif tn in ("InstMatmult", "InstLdweights"):
                        assert ldw_tpl is not None, "no ldweights template"
                        nops = [make_nop(ldw_tpl, w) for w in waits[:-1]]
                        # keep split LDW+MM pairs adjacent
                        if out and type(out[-1]).__name__ == "InstLdweights":
                            own_ldw = out.pop()
                            nops.append(own_ldw)
                    elif tn == "InstDrain":
                        # a drain is its own carrier: extra single-wait drains
                        # on the same engine are harmless
                        nops = [make_nop(inst, w) for w in waits[:-1]]
                    elif inst.engine in eng_tpl and tn not in (
                        "InstDrain",
                        "InstEventSemaphore",
                        "InstSemaphoreOp",
                    ):
                        nops = [make_nop(eng_tpl[inst.engine], w) for w in waits[:-1]]
                if nops:
                    out.extend(nops)
                    inst.sync_info = mybir.SyncInfo(
                        on_wait=waits[-1:], on_update=list(si.on_update)
                    )
                    changed = True
                out.append(inst)
            if changed:
                try:
                    blk.instructions[:] = out
                except TypeError:
                    blk.instructions.clear()
                    blk.instructions.extend(out)


_PROGRAM_CACHE: dict = {}


def _get_program(n_loc_pad: int, n_src: int) -> bass.Bass:
    key = (n_loc_pad, n_src)
    if key not in _PROGRAM_CACHE:
        _PROGRAM_CACHE[key] = build_program(n_loc_pad, n_src)
    return _PROGRAM_CACHE[key]


def prep_gather(nh_pad: np.ndarray):
    """Bucket edges by nh%4 per tile, emit int16 super-row ids (wrapped
    [16, L/16] layout replicated to 128 partitions) and per-tile slot-owner
    vectors for the on-device one-hot build.

    Returns (idx16 [128, n_groups*NCLS*seg_i16], owner [128, n_tiles*CH_T] bf16).
    """
    import ml_dtypes

    n_pad = nh_pad.shape[0]
    n_tiles = n_pad // P
    n_groups = n_tiles // GRP
    seg_i16 = GRP * SEG // 16

    idx16 = np.zeros((n_groups * NCLS, GRP * SEG), np.int16)
    owner = np.full((n_tiles, NCLS * SEG), -1.0, np.float32)  # [tile, slot]
    for t in range(n_tiles):
        nh_t = nh_pad[t * P:(t + 1) * P]          # [128 nodes, K]
        nodes = np.repeat(np.arange(P), K)         # edge -> node
        vals = nh_t.reshape(-1)                    # edge -> neighbor id
        cls = vals % NCLS
        g, ti = divmod(t, GRP)
        for j in range(NCLS):
            sel = np.nonzero(cls == j)[0]
            l = len(sel)
            assert l <= SEG, f"class overflow {l} > {SEG}"
            idx16[g * NCLS + j, ti * SEG:ti * SEG + l] = (vals[sel] // NCLS).astype(
                np.int16
            )
            # slot s of class j occupies pool chunk row (j*SEG + s)
            owner[t, j * SEG + np.arange(l)] = nodes[sel]
    # wrap idx16: entry i -> [i%16, i//16]; replicate 16-row block to 128
    idx16 = idx16.reshape(n_groups * NCLS, GRP * SEG // 16, 16).transpose(0, 2, 1)
    idx16 = np.tile(idx16, (1, 8, 1)).reshape(n_groups, NCLS, P, seg_i16)
    idx16 = np.ascontiguousarray(
        idx16.transpose(2, 0, 1, 3).reshape(P, n_groups * NCLS * seg_i16)
    )
    # owner: [tile, slot] -> [slot%128 partitions, tile*CH_T + slot//128]
    owner = owner.reshape(n_tiles, CH_T, P).transpose(2, 0, 1)
    owner = np.ascontiguousarray(
        owner.reshape(P, n_tiles * CH_T)
    ).astype(ml_dtypes.bfloat16)
    return idx16, owner


def assemble_out(res_core: dict, n_tiles: int) -> np.ndarray:
    """Per-chunk partition-major bf16 outputs -> [n_loc_pad, 128] f32."""
    parts = []
    for c, ct in enumerate(_chunks(n_tiles)):
        o = np.asarray(res_core[f"out{c}"]).astype(np.float32)  # [128, ct*128]
        parts.append(
            o.reshape(P, ct, XN_OUT).transpose(1, 0, 2).reshape(ct * P, XN_OUT)
        )
    return np.concatenate(parts, axis=0)


def make_in_maps(x, e, ij, Wc, Wn, We, n_cores=N_CORES):
    """Host-side shard/prep: per-core input dicts for the SPMD program."""
    import ml_dtypes

    n = x.shape[0]
    n_loc = n // n_cores
    n_loc_pad = ((n_loc + P - 1) // P) * P

    x_bf = np.ascontiguousarray(x).astype(ml_dtypes.bfloat16)
    nh = np.ascontiguousarray(ij[:, :, 0]).astype(np.int32)
    wcT = np.ascontiguousarray(Wc.T).astype(ml_dtypes.bfloat16)
    wnT = (np.ascontiguousarray(Wn.T) / np.float32(K)).astype(ml_dtypes.bfloat16)
    weT = (np.ascontiguousarray(We.T) / np.float32(K)).astype(ml_dtypes.bfloat16)
    iota = np.broadcast_to(
        np.arange(P, dtype=np.float32), (P, P)
    ).astype(ml_dtypes.bfloat16)

    in_maps = []
    for c in range(n_cores):
        sl = slice(c * n_loc, (c + 1) * n_loc)
        x_selfT = np.zeros((XN_IN, n_loc_pad), ml_dtypes.bfloat16)
        x_selfT[:, :n_loc] = x_bf[sl].T
        e_locT = np.zeros((XE_IN, n_loc_pad * K), ml_dtypes.bfloat16)
        # e[sl] is [n_loc, K, 64] -> [64, n_loc, K] feature-major, k contiguous
        e_locT[:, :n_loc * K] = (
            np.asarray(e[sl], np.float32)
            .transpose(2, 0, 1)
            .reshape(XE_IN, n_loc * K)
            .astype(ml_dtypes.bfloat16)
        )
        # pad rows cycle 0..3 so no per-tile mod-class bucket overflows SEG
        idx_c = np.tile(np.arange(K, dtype=np.int32) % NCLS, (n_loc_pad, 1))
        idx_c[:n_loc] = nh[sl]
        idx16, owner = prep_gather(idx_c)
        in_maps.append(
            {
                "x_bf": x_bf,
                "x_selfT": x_selfT,
                "e_locT": e_locT,
                "idx_loc": idx16,
                "owner_loc": owner,
                "iota_in": iota,
                "wcT": wcT,
                "wnT": wnT,
                "weT": weT,
            }
        )
    return in_maps, n_loc, n_loc_pad


def kernel(x, e, ij, Wc, Wn, We):
    x = np.asarray(x)
    e = np.asarray(e)
    ij = np.asarray(ij)
    in_maps, n_loc, n_loc_pad = make_in_maps(x, e, ij, Wc, Wn, We)
    nc = _get_program(n_loc_pad, x.shape[0])
    res = run_bass_kernel_spmd(nc, in_maps, list(range(N_CORES)))
    n_tiles = n_loc_pad // P
    out = np.concatenate(
        [assemble_out(r, n_tiles)[:n_loc] for r in res.results], axis=0
    )
    return out.astype(np.float32)


# revision 4
# speedup vs baseline: 75.1181x; 75.1181x over previous
"""Trainium2 Bass kernel: GNN conv block (nn_Conv_block_49331994362308).

Computes, for N=100000 nodes with K=16 neighbors each:
    nh  = ij[:, :, 0]                      # [N, K] neighbor ids
    xnj = mean(x[nh], axis=1)              # neighbor-feature mean  [N, 128]
    xej = mean(e, axis=1)                  # edge-feature mean      [N, 64]
    out = relu(x @ Wc.T + xnj @ Wn.T + xej @ We.T)

Distribution: data-parallel over nodes across 8 NeuronCores (12500 nodes
per core, padded to 12544 = 98*128). x is replicated to every core so the
random neighbor gather x[nh] is a core-local indirect DMA from HBM.

The hard serial resource is the SWDGE gather ucode on GpSimd: measured
~7.4ns per index ENTRY (flat in element size, -1 entries, and batching),
so the whole kernel is structured to keep every other engine far below
that wall and the gather stream never stalled:
  - Neighbor rows are gathered from a bf16 copy of x (256B rows) via
    InstDMAGatherAnt, one instruction per mod-4 row class per 7-tile
    group (int16 indices -> x viewed as [N/4, 4, 128] super-rows, host
    buckets edges by nh%4, pads per-tile buckets to SEG=640 slots).
  - The one-hot pooling matrices P[slot, node] are host-built fp8 and
    DMA'd (DMA bandwidth has ~5x slack; building them on DVE measured
    16us/tile). PE pools gathered rows with 20 bf16x fp8 matmuls/tile
    into fp32 PSUM; 1/K is folded into Wn/We on the host.
  - The e-mean is folded into PE: e is host-transposed to kf-major
    [128, 8, nodes] bf16 and contracted with [We.T; We.T]/K in 8
    accumulating matmuls directly into the output PSUM (a DVE reduce
    measured 16us/tile; PE is ~90% idle).
  - ACT does the two PSUM->SBUF hops (xnjT copy + final ReLU, both with
    bf16 cast); DVE runs nothing in steady state. Output is bf16,
    upcast on the host.

Walrus's TRN2 queue-DMA codegen only supports ONE sync-wait command per
DMA (and one per PE LDWEIGHTS), so the structure keeps every DMA at a
single dependency front: indices are preloaded once into SBUF (gathers
then wait only on the PE pool-slot release), the 8 SWDGE bookkeeping
lanes are warmed with dummy transfers that absorb the preload front, and
outputs go to once-written per-chunk DRAM tensors (no WAW chains).
"""

from contextlib import ExitStack

import numpy as np

import concourse.bass as bass
import concourse.mybir as mybir
import concourse.tile as tile
from concourse.bass_utils import run_bass_kernel_spmd
from concourse import library_config

P = 128
K = 16
XN_IN = 128
XE_IN = 64
XN_OUT = 128
N_CORES = 8
N_FULL = 100000
N_LOC = N_FULL // N_CORES          # 12500
N_LOC_PAD = ((N_LOC + P - 1) // P) * P  # 12544
CHUNK = 14                          # tiles per output chunk (98 = 7*14)

F32 = mybir.dt.float32
BF16 = mybir.dt.bfloat16
F8 = mybir.dt.float8e4   # pooling matrices hold only 0/1 — exact in fp8
I16 = mybir.dt.int16

GRP = 7            # tiles per gather group
NCLS = 4           # x rows per int16 "super-row" (mod classes)
SEG = 640          # padded gather slots per (tile, class); 5 chunks of 128
NCH = SEG // P     # pool chunks per (tile, class) = 5
CH_T = NCH * NCLS  # pool chunks per tile = 20
ECH = K * XE_IN // P  # e contraction chunks per tile = 8


def _chunks(n_tiles: int) -> list[int]:
    out = []
    t = 0
    while t < n_tiles:
        out.append(min(CHUNK, n_tiles - t))
        t += CHUNK
    return out


def build_program(n_loc_pad: int, n_src: int) -> bass.Bass:
    """Build the SPMD per-core Bass program (same program on every core)."""
    assert n_loc_pad % P == 0
    n_tiles = n_loc_pad // P
    chunks = _chunks(n_tiles)

    # detect_race_conditions=False: the post-schedule wait-legalizer's nop
    # carriers share scratch tiles and trip the sim race detector's
    # bookkeeping (same-engine program order makes them safe).
    nc = bass.Bass("TRN2", debug=False, detect_race_conditions=False)

    assert n_tiles % GRP == 0
    n_groups = n_tiles // GRP
    seg_i16 = GRP * SEG // 16  # idx16 columns per (group, class)

    x_bf = nc.dram_tensor("x_bf", [n_src, XN_IN], BF16, kind="ExternalInput").ap()
    # x_self pre-transposed: [128 feat, nodes] bf16
    x_selfT = nc.dram_tensor("x_selfT", [XN_IN, n_loc_pad], BF16, kind="ExternalInput").ap()
    # e in kf-major chunks: e_pe[p, c*n_loc_pad + n] = e[n, kf//64, kf%64],
    # kf = c*128 + p
    e_pe = nc.dram_tensor("e_pe", [P, ECH * n_loc_pad], BF16, kind="ExternalInput").ap()
    # int16 super-row ids (nh//4), wrapped [16, L/16] + replicated to 128
    # partitions, concatenated over (group, class)
    idx_loc = nc.dram_tensor(
        "idx_loc", [P, n_groups * NCLS * seg_i16], I16, kind="ExternalInput"
    ).ap()
    # pooling one-hot matrices, per tile [128 slots, CH_T*128 nodes] fp8
    pool_loc = nc.dram_tensor(
        "pool_loc", [P, n_tiles * CH_T * P], F8, kind="ExternalInput"
    ).ap()
    wcT = nc.dram_tensor("wcT", [XN_IN, XN_OUT], BF16, kind="ExternalInput").ap()
    wnT = nc.dram_tensor("wnT", [XN_IN, XN_OUT], BF16, kind="ExternalInput").ap()
    # [We.T; We.T]/K — identical moving operand for all 8 e-chunks
    we2 = nc.dram_tensor("we2", [P, XN_OUT], BF16, kind="ExternalInput").ap()
    # per-chunk outputs, partition-major: out_c[p, i*128+f] = out[(t0+i)*128+p, f]
    outs = [
        nc.dram_tensor(f"out{c}", [P, ct * XN_OUT], BF16, kind="ExternalOutput").ap()
        for c, ct in enumerate(chunks)
    ]

    nop_sem = nc.alloc_semaphore("waitnop")

    with tile.TileContext(nc) as tc, ExitStack() as ctx:
        nc.gpsimd.sem_clear(range(nop_sem.num, nop_sem.num + 1))
        nc.gpsimd.load_library(library_config.mlp)
        consts = ctx.enter_context(tc.tile_pool(name="consts", bufs=1))
        wcT_sb = consts.tile([XN_IN, XN_OUT], BF16, tag="wc")
        wnT_sb = consts.tile([XN_IN, XN_OUT], BF16, tag="wn")
        we2_sb = consts.tile([P, XN_OUT], BF16, tag="we2")
        nc.sync.dma_start(wcT_sb[:], wcT[:, :])
        nc.sync.dma_start(wnT_sb[:], wnT[:, :])
        nc.sync.dma_start(we2_sb[:], we2[:, :])
        idx_all = consts.tile([P, n_groups * NCLS * seg_i16], I16, tag="idx_all")
        nc.sync.dma_start(idx_all[:], idx_loc[:, :])
        # x viewed as [n_src/4, 4, 128]: class j gathers row 4*i16+j via
        # elem_step=512 elements (1024B stride) and a j*128-element offset
        x4 = x_bf.rearrange("(r c) f -> r c f", c=NCLS)

        # Warm the 8 SWDGE bookkeeping lanes: each dummy absorbs the
        # idx-preload front so later gathers carry only their PE front.
        scratch = ctx.enter_context(tc.tile_pool(name="scratch", bufs=1))
        for q in range(8):
            sc = scratch.tile([1, K], I16, tag=f"sc{q}")
            nc.gpsimd.dma_start(sc[:], idx_all[:1, :K])
        # Tiny template instructions for _legalize_waits nop carriers
        # (one per DMA queue and per compute engine).
        nop_hw = scratch.tile([1, K], I16, tag="noptpl_hw")
        nc.sync.dma_start(nop_hw[:], idx_loc[:1, :K])
        nop_sw = scratch.tile([1, K], I16, tag="noptpl_sw")
        nc.gpsimd.dma_start(nop_sw[:], idx_loc[:1, :K])
        nop_dve = scratch.tile([P, K], BF16, tag="noptpl_dve")
        nc.vector.tensor_copy(nop_dve[:], wcT_sb[:, :K])
        nop_act = scratch.tile([P, K], BF16, tag="noptpl_act")
        nc.scalar.copy(nop_act[:], wcT_sb[:, :K])
        nop_pool = scratch.tile([P, K], F32, tag="noptpl_pool")
        nc.gpsimd.memset(nop_pool[:], 0.0)

        g_pool = ctx.enter_context(tc.tile_pool(name="gatherp", bufs=3))
        pp_pool = ctx.enter_context(tc.tile_pool(name="poolmat", bufs=3))
        e_pool = ctx.enter_context(tc.tile_pool(name="edgep", bufs=4))
        xs_pool = ctx.enter_context(tc.tile_pool(name="xselfp", bufs=4))
        st_pool = ctx.enter_context(tc.tile_pool(name="stagep", bufs=3))
        out_pool = ctx.enter_context(tc.tile_pool(name="outp", bufs=2))
        psum_pool = ctx.enter_context(tc.tile_pool(name="psump", bufs=2, space="PSUM"))
        psum1_pool = ctx.enter_context(tc.tile_pool(name="psum1p", bufs=1, space="PSUM"))

        # Warm up PE's view of the constants so steady-state matmuls carry at
        # most one sync wait (PE LDWEIGHTS supports a single wait command).
        ps_warm = psum1_pool.tile([P, P], F32, tag="warm")
        nc.tensor.matmul(ps_warm[:], wcT_sb[:], wcT_sb[:], start=True, stop=False)
        nc.tensor.matmul(ps_warm[:], wnT_sb[:], wnT_sb[:], start=False, stop=False)
        nc.tensor.matmul(ps_warm[:], we2_sb[:], we2_sb[:], start=False, stop=True)

        t = 0
        gbf = [None] * NCLS
        nidx_reg = nc.gpsimd.to_reg(GRP * SEG)  # shared across all gathers
        relu = mybir.ActivationFunctionType.Relu
        for c, ct in enumerate(chunks):
            o_stage = out_pool.tile([P, ct * XN_OUT], BF16, tag="ostage")
            for i in range(ct):
                g, ti = divmod(t, GRP)

                if ti == 0:
                    # per-group gathers: one dma_gather per mod-4 class of
                    # GRP*SEG slots; slot i lands at partition i%128, free
                    # block i//128, so 128-slot chunks stay within one tile.
                    for j in range(NCLS):
                        off = (g * NCLS + j) * seg_i16
                        gb = g_pool.tile(
                            [P, GRP * SEG // P, XN_IN], BF16, tag=f"go{j}"
                        )
                        nc.gpsimd.dma_gather(
                            out_ap=gb[:],
                            in_ap=x4[:, j, :],
                            idxs_ap=idx_all[:, off:off + seg_i16],
                            num_idxs=GRP * SEG,
                            num_idxs_reg=nidx_reg,
                            elem_size=XN_IN,
                            elem_step=NCLS * XN_IN,
                            single_packet=False,
                        )
                        gbf[j] = gb

                x_sb = xs_pool.tile([XN_IN, P], BF16, tag="xs")
                nc.sync.dma_start(x_sb[:], x_selfT[:, t * P:(t + 1) * P])
                e_sb = e_pool.tile([P, ECH, P], BF16, tag="e")
                nc.sync.dma_start(
                    e_sb[:],
                    e_pe.rearrange("p (c n) -> p c n", c=ECH)[
                        :, :, t * P:(t + 1) * P
                    ],
                )
                p_sb = pp_pool.tile([P, CH_T * P], F8, tag="pmat")
                nc.sync.dma_start(
                    p_sb[:], pool_loc[:, t * CH_T * P:(t + 1) * CH_T * P]
                )

                # xnjT[f, n] = sum_slot g[slot, f] * P[slot, n]
                xnjT_ps = psum_pool.tile([P, P], F32, tag="ps_xnj")
                for b in range(CH_T):
                    j, bl = divmod(b, NCH)
                    blk = ti * NCH + bl
                    nc.tensor.matmul(
                        xnjT_ps[:],
                        gbf[j][:, blk, :],
                        p_sb[:, b * P:(b + 1) * P],
                        start=(b == 0),
                        stop=(b == CH_T - 1),
                    )

                # xnjT PSUM -> SBUF (+ bf16 cast) on ACT
                xnjT_sb = st_pool.tile([P, P], BF16, tag="sb_xnj")
                nc.scalar.copy(xnjT_sb[:], xnjT_ps[:])

                # out[n, fo] = sum_kf e[kf, n]*we2[kf%128, fo]
                #            + sum_f x[f, n]*wcT[f, fo] + xnjT[f, n]*wnT[f, fo]
                out_ps = psum_pool.tile([P, XN_OUT], F32, tag="ps_out")
                for ec in range(ECH):
                    nc.tensor.matmul(
                        out_ps[:], e_sb[:, ec, :], we2_sb[:],
                        start=(ec == 0), stop=False,
                    )
                nc.tensor.matmul(out_ps[:], x_sb[:], wcT_sb[:], start=False, stop=False)
                nc.tensor.matmul(out_ps[:], xnjT_sb[:], wnT_sb[:], start=False, stop=True)

                # ReLU (+ bf16 cast) on ACT into the chunk staging buffer
                nc.scalar.activation(
                    o_stage[:, i * XN_OUT:(i + 1) * XN_OUT], out_ps[:], relu
                )
                t += 1

            nc.sync.dma_start(outs[c][:, :], o_stage[:])

    from concourse.library_overlay import lower_extended_insts

    lower_extended_insts(nc)
    _legalize_waits(nc, nop_sem)
    return nc


def _legalize_waits(nc: bass.Bass, nop_sem) -> None:
    """Split multi-wait queue-DMAs / matmuls for walrus's 1-wait codegen limit.

    The TRN2 walrus codegen allows a single sync-wait command per queue-DMA
    entry and per PE matmul (S3_LW struct). Tile emits minimal waits but can
    still produce 2+ (e.g. a slot's previous-writer DMA completion plus its
    last-reader engine release — Tile's clocks are not transitive). Queue
    entries execute in FIFO order, so extra waits are moved onto tiny no-op
    carrier DMAs inserted immediately before the offender on the same queue.
    For matmuls the carrier is a 1-column bf16 LDWEIGHTS (any clobbered
    weights are reloaded by each matmul's own weight load; insertion happens
    before a directly-preceding LDWEIGHTS so split LDW+MM pairs stay intact).
    """
    import copy

    dma_tpl: dict = {}
    eng_tpl: dict = {}
    evsem_tpl: dict = {}
    ldw_tpl = None
    for f in nc.m.functions:
        for blk in f.blocks:
            for inst in blk.instructions:
                tn = type(inst).__name__
                dst = (
                    str(getattr(inst.outs[0], "memref", "")) if inst.outs else ""
                )
                if tn == "InstDMACopy":
                    if dst.startswith("nop_hw"):
                        dma_tpl["qSPDynamicHW"] = inst
                    elif dst.startswith("nop_sw"):
                        dma_tpl[inst.queue] = inst
                elif tn == "InstLdweights" and ldw_tpl is None:
                    ldw_tpl = inst
                elif tn == "InstEventSemaphore":
                    evsem_tpl[inst.engine] = inst
                elif dst.startswith("nop_dve") or dst.startswith("nop_act") or dst.startswith("nop_pool"):
                    eng_tpl[inst.engine] = inst

    counter = [0]

    def make_nop(tpl, wait):
        counter[0] += 1
        nop = copy.deepcopy(tpl)
        nop.name = f"I-{nc.next_id()}"
        # DMA carriers must update a semaphore (BIR invariant); use a
        # dedicated one nobody waits on. Other engines' carriers stay
        # update-free (walrus rejects a waitnop update on e.g. TensorCopy
        # with a no_semaphore_value_conflict ISA check).
        upd = []
        if type(tpl).__name__ == "InstDMACopy":
            upd = [
                mybir.SyncUpdate(
                    sync_type="semaphore",
                    id=nop_sem.num,
                    ant_name=nop_sem.name,
                    update_mode="sem-add-imm",
                    update_value=16,
                )
            ]
        nop.sync_info = mybir.SyncInfo(on_wait=[wait], on_update=upd)
        nc.inst_map[nop.name] = nop
        return nop

    for f in nc.m.functions:
        for blk in f.blocks:
            out: list = []
            changed = False
            insts = list(blk.instructions)
            for pos, inst in enumerate(insts):
                tn = type(inst).__name__
                si = inst.sync_info
                waits = list(si.on_wait) if si else []
                nops = None
                if len(waits) > 1:
                    if tn == "InstDMACopy":
                        tpl = dma_tpl.get(inst.queue)
                        assert tpl is not None, f"no nop template for {inst.queue}"
                        nops = [make_nop(tpl, w) for w in waits[:-1]]
                    elif tn in ("InstMatmult", "InstLdweights"):
                        assert ldw_tpl is not None, "no ldweights template"
                        nops = [make_nop(ldw_tpl, w) for w in waits[:-1]]
                        # keep split LDW+MM pairs adjacent
                        if out and type(out[-1]).__name__ == "InstLdweights":
                            own_ldw = out.pop()
                            nops.append(own_ldw)
                    elif tn == "InstDrain":
                        # a drain is its own carrier: extra single-wait drains
                        # on the same engine are harmless
                        nops = [make_nop(inst, w) for w in waits[:-1]]
                    elif inst.engine in eng_tpl and tn not in (
                        "InstDrain",
                        "InstEventSemaphore",
                        "InstSemaphoreOp",
                    ):
                        nops = [make_nop(eng_tpl[inst.engine], w) for w in waits[:-1]]
                if nops:
                    out.extend(nops)
                    inst.sync_info = mybir.SyncInfo(
                        on_wait=waits[-1:], on_update=list(si.on_update)
                    )
                    changed = True
                out.append(inst)
            if changed:
                try:
                    blk.instructions[:] = out
                except TypeError:
                    blk.instructions.clear()
                    blk.instructions.extend(out)


_PROGRAM_CACHE: dict = {}


def _get_program(n_loc_pad: int, n_src: int) -> bass.Bass:
    key = (n_loc_pad, n_src)
    if key not in _PROGRAM_CACHE:
        _PROGRAM_CACHE[key] = build_program(n_loc_pad, n_src)
    return _PROGRAM_CACHE[key]


def prep_gather(nh_pad: np.ndarray):
    """Bucket edges by nh%4 per tile, emit int16 super-row ids (wrapped
    [16, L/16] layout replicated to 128 partitions) and per-tile one-hot
    pooling matrices.

    Returns (idx16 [128, n_groups*NCLS*seg_i16], pool [128, n_tiles*CH_T*128] fp8).
    """
    import ml_dtypes

    n_pad = nh_pad.shape[0]
    n_tiles = n_pad // P
    n_groups = n_tiles // GRP
    seg_i16 = GRP * SEG // 16

    idx16 = np.zeros((n_groups * NCLS, GRP * SEG), np.int16)
    pool = np.zeros((n_tiles, CH_T * P, P), np.float32)  # [tile, slot, node]
    for t in range(n_tiles):
        nh_t = nh_pad[t * P:(t + 1) * P]          # [128 nodes, K]
        nodes = np.repeat(np.arange(P), K)         # edge -> node
        vals = nh_t.reshape(-1)                    # edge -> neighbor id
        cls = vals % NCLS
        g, ti = divmod(t, GRP)
        for j in range(NCLS):
            sel = np.nonzero(cls == j)[0]
            l = len(sel)
            assert l <= SEG, f"class overflow {l} > {SEG}"
            idx16[g * NCLS + j, ti * SEG:ti * SEG + l] = (vals[sel] // NCLS).astype(
                np.int16
            )
            # slot s of class j occupies pool chunk row (j*SEG + s)
            pool[t, j * SEG + np.arange(l), nodes[sel]] = 1.0
    # wrap idx16: entry i -> [i%16, i//16]; replicate 16-row block to 128
    idx16 = idx16.reshape(n_groups * NCLS, GRP * SEG // 16, 16).transpose(0, 2, 1)
    idx16 = np.tile(idx16, (1, 8, 1)).reshape(n_groups, NCLS, P, seg_i16)
    idx16 = np.ascontiguousarray(
        idx16.transpose(2, 0, 1, 3).reshape(P, n_groups * NCLS * seg_i16)
    )
    # pool: [tile, slot(CH_T*128), node] -> chunk layout [slot%128, tile, chunk, node]
    pool = pool.reshape(n_tiles, CH_T, P, P).transpose(2, 0, 1, 3)
    pool = np.ascontiguousarray(
        pool.reshape(P, n_tiles * CH_T * P)
    ).astype(ml_dtypes.float8_e4m3)
    return idx16, pool


def assemble_out(res_core: dict, n_tiles: int) -> np.ndarray:
    """Per-chunk partition-major bf16 outputs -> [n_loc_pad, 128] f32."""
    parts = []
    for c, ct in enumerate(_chunks(n_tiles)):
        o = np.asarray(res_core[f"out{c}"]).astype(np.float32)  # [128, ct*128]
        parts.append(
            o.reshape(P, ct, XN_OUT).transpose(1, 0, 2).reshape(ct * P, XN_OUT)
        )
    return np.concatenate(parts, axis=0)


def make_in_maps(x, e, ij, Wc, Wn, We, n_cores=N_CORES):
    """Host-side shard/prep: per-core input dicts for the SPMD program."""
    import ml_dtypes

    n = x.shape[0]
    n_loc = n // n_cores
    n_loc_pad = ((n_loc + P - 1) // P) * P

    x_bf = np.ascontiguousarray(x).astype(ml_dtypes.bfloat16)
    nh = np.ascontiguousarray(ij[:, :, 0]).astype(np.int32)
    wcT = np.ascontiguousarray(Wc.T).astype(ml_dtypes.bfloat16)
    wnT = (np.ascontiguousarray(Wn.T) / np.float32(K)).astype(ml_dtypes.bfloat16)
    weT = np.ascontiguousarray(We.T) / np.float32(K)
    we2 = np.ascontiguousarray(np.vstack([weT, weT])).astype(ml_dtypes.bfloat16)

    in_maps = []
    for c in range(n_cores):
        sl = slice(c * n_loc, (c + 1) * n_loc)
        x_selfT = np.zeros((XN_IN, n_loc_pad), ml_dtypes.bfloat16)
        x_selfT[:, :n_loc] = x_bf[sl].T
        # e[sl] [n_loc, K, 64] -> kf-major [ECH*128, n] -> [128, ECH, n]
        e_pe = np.zeros((P, ECH, n_loc_pad), ml_dtypes.bfloat16)
        e_kfn = (
            np.asarray(e[sl], np.float32).reshape(n_loc, K * XE_IN).T
        )  # [1024, n_loc]
        e_pe[:, :, :n_loc] = (
            e_kfn.reshape(ECH, P, n_loc).transpose(1, 0, 2)
        ).astype(ml_dtypes.bfloat16)
        # pad rows cycle 0..3 so no per-tile mod-class bucket overflows SEG
        idx_c = np.tile(np.arange(K, dtype=np.int32) % NCLS, (n_loc_pad, 1))
        idx_c[:n_loc] = nh[sl]
        idx16, pool_m = prep_gather(idx_c)
        in_maps.append(
            {
                "x_bf": x_bf,
                "x_selfT": x_selfT,
                "e_pe": e_pe.reshape(P, ECH * n_loc_pad),
                "idx_loc": idx16,
                "pool_loc": pool_m,
                "wcT": wcT,
                "wnT": wnT,
                "we2": we2,
            }
        )
    return in_maps, n_loc, n_loc_pad


def kernel(x, e, ij, Wc, Wn, We):
    x = np.asarray(x)
    e = np.asarray(e)
    ij = np.asarray(ij)
    in_maps, n_loc, n_loc_pad = make_in_maps(x, e, ij, Wc, Wn, We)
    nc = _get_program(n_loc_pad, x.shape[0])
    res = run_bass_kernel_spmd(nc, in_maps, list(range(N_CORES)))
    n_tiles = n_loc_pad // P
    out = np.concatenate(
        [assemble_out(r, n_tiles)[:n_loc] for r in res.results], axis=0
    )
    return out.astype(np.float32)
